# revision 11
# baseline (speedup 1.0000x reference)
"""Trainium2 Bass kernel for nn_CustomNetwork (4-layer 4096x4096 MLP with
train-mode BatchNorm1d + ReLU per layer, batch-axis softmax at the end).

Strategy: data-parallel over the batch dim across 8 NeuronCores (512 rows
per core). Activations live in SBUF transposed (channels on partitions,
batch on the free dim) so BatchNorm stats and the batch-axis softmax are
native free-axis reductions. Matmuls run in fp16 (same PE rate as
fp32r/bf16 but half the weight DMA, fast weight load, and 8x finer
mantissa than bf16 -- platform power management pins the PE near 2.0 GHz
either way, so fp16's extra precision is free). BatchNorm statistics and
softmax sums stay in fp32. Weights are host-retiled to [L, KT, NSUP, P,
512] so every weight-tile DMA is one 128 KiB contiguous block. PSUM is
managed as eight independent single-bank tiles so accumulation-group
dependencies stay per-bank.

Cross-core BatchNorm mean/var and the softmax exp-sum use AllReduce over
small per-channel vectors. Each collective occupies the CC core for
~10-20us plus ~5us of trigger latency, so the schedule keeps the count
low and issues each one as early as its inputs allow:
  - mid layers use three stat chunks [0,24)/[24,28)/[28,32) so the last
    chunk's allreduce round-trip hides under the next layer's first
    24 k-steps;
  - the last layer splits the early BN chunk in two ([0,16) at 4/8 of
    the layer, [16,24) at 6/8) so all 24 early exp tiles AND their
    softmax-sum allreduce complete before the final matmul, and runs its
    final supertile tile-major (weights preloaded) so only one tile's
    stats trail the last matmul. After the final matmul only two small
    [P,2,8]/[P,8] collectives remain, and the normalize+store of tiles
    0..23 hides under the first of them.

Note: the Linear bias `b` is mathematically canceled by BatchNorm's mean
subtraction, so it is never loaded.
"""

import numpy as np

import concourse.bacc as bacc
import concourse.mybir as mybir
import concourse.tile as tile
from concourse import bass_utils

P = 128  # SBUF partitions
D = 4096  # feature width
KT = D // P  # 32 k/n tiles
BM = 512  # per-core batch (4096 / 8 cores)
NSUP = 8  # n supertiles of 512 output channels
L = 4  # layers
N_CORES = 8
BN_EPS = 1e-5
# BN-stat allreduce chunks (mid layers).
CHUNKS = [(0, 24), (24, 28), (28, 32)]
# last layer: early chunks split further so each chunk's allreduce lands
# while the DVE queue is free and all 24 early exp tiles plus their
# softmax-sum allreduce finish before the final matmul
CHUNKS_LAST = [(0, 16), (16, 20), (20, 24)]

F32 = mybir.dt.float32
F16 = mybir.dt.float16

_cached_nc = None


def _bn_scale_shift(nc, small, red, gam_ap, bet_ap, n, tag):
    """From allreduced [P, 2, n] (sum of means, sum of E[h^2]) compute
    scale = gamma/sqrt(var+eps), shift = beta - mean*scale."""
    mean_g = small.tile([P, n], F32, name=f"mean_{tag}")
    var_g = small.tile([P, n], F32, name=f"var_{tag}")
    scale = small.tile([P, n], F32, name=f"scale_{tag}")
    shift = small.tile([P, n], F32, name=f"shift_{tag}")
    nc.vector.tensor_scalar_mul(mean_g[:], red[:, 0, :], 1.0 / N_CORES)
    nc.vector.tensor_scalar_mul(var_g[:], red[:, 1, :], 1.0 / N_CORES)
    # var = E[h^2] - mean^2
    nc.vector.tensor_tensor(scale[:], mean_g[:], mean_g[:], op=mybir.AluOpType.mult)
    nc.vector.tensor_sub(var_g[:], var_g[:], scale[:])
    nc.vector.tensor_scalar_add(var_g[:], var_g[:], BN_EPS)
    nc.scalar.activation(
        scale[:], var_g[:], mybir.ActivationFunctionType.Sqrt, bias=0.0, scale=1.0
    )
    nc.vector.reciprocal(scale[:], scale[:])
    nc.vector.tensor_mul(scale[:], scale[:], gam_ap)
    nc.vector.tensor_tensor(shift[:], mean_g[:], scale[:], op=mybir.AluOpType.mult)
    nc.vector.tensor_sub(shift[:], bet_ap, shift[:])
    return scale, shift


def _pack_stats(nc, small, meanvar, t0, t1, tag):
    """pack[:,0,:] = local mean; pack[:,1,:] = E[h^2] = var + mean^2."""
    n = t1 - t0
    pack = small.tile([P, 2, n], F32, name=f"pack_{tag}")
    nc.vector.tensor_copy(pack[:, 0, :], meanvar[:, t0:t1, 0])
    nc.vector.tensor_tensor(
        pack[:, 1, :], meanvar[:, t0:t1, 0], meanvar[:, t0:t1, 0],
        op=mybir.AluOpType.mult,
    )
    nc.vector.tensor_tensor(
        pack[:, 1, :], pack[:, 1, :], meanvar[:, t0:t1, 1], op=mybir.AluOpType.add
    )
    return pack


def build():
    global _cached_nc
    if _cached_nc is not None:
        return _cached_nc
    nc = bacc.Bacc("TRN2", target_bir_lowering=False, debug=False, num_devices=N_CORES)

    xt = nc.dram_tensor("xt", [D, BM], F16, kind="ExternalInput")
    # host-retiled weights: [l, k, ns] tile is a contiguous [P, 512] block
    Wt = nc.dram_tensor("W", [L, KT, NSUP, P, 512], F16, kind="ExternalInput")
    # gammaH/betaH are host-transposed to [L, P, KT] so the DMA runs with
    # contiguous lines
    gamma = nc.dram_tensor("gammaH", [L, P, KT], F32, kind="ExternalInput")
    beta = nc.dram_tensor("betaH", [L, P, KT], F32, kind="ExternalInput")
    outt = nc.dram_tensor("outt", [D, BM], F16, kind="ExternalOutput")

    rg = [list(range(N_CORES))]

    def allreduce(pool_dram, src_ap, dst_tile, tag, eng=None):
        # eng picks the DGE queue for the staging DMAs; the post-matmul
        # collectives use the Activation queue so their results don't wait
        # behind bulk weight/store traffic on the SP queue.
        eng = eng or nc.sync
        ar_in = pool_dram.tile(list(src_ap.shape), F32, name=f"arin_{tag}")
        ar_out = pool_dram.tile(list(src_ap.shape), F32, name=f"arout_{tag}")
        eng.dma_start(ar_in[:], src_ap)
        nc.gpsimd.collective_compute(
            "AllReduce",
            mybir.AluOpType.add,
            replica_groups=rg,
            ins=[ar_in.opt()],
            outs=[ar_out.opt()],
        )
        eng.dma_start(dst_tile[:], ar_out[:])

    with tile.TileContext(nc) as tc:
        with (
            tc.tile_pool(name="hbuf", bufs=1) as hpool,
            tc.tile_pool(name="wpool", bufs=48) as wpool,
            tc.tile_pool(name="w7", bufs=1) as w7pool,
            tc.tile_pool(name="psum", bufs=8, space="PSUM") as psum,
            tc.tile_pool(name="small", bufs=2) as small,
            tc.tile_pool(name="gb", bufs=1) as gbpool,
            tc.tile_pool(name="dram", bufs=1, space="DRAM") as dram,
        ):
            h = [
                hpool.tile([P, KT, BM], F16, name="h_a"),
                hpool.tile([P, KT, BM], F16, name="h_b"),
            ]

            # x^T -> h[0] on the Activation-engine DGE queue (idle at start)
            # while the SP queue streams layer-0 weights, so the two
            # prefetch streams don't serialize behind each other.
            w_pre = []
            for k in range(KT):
                nc.scalar.dma_start(h[0][:, k, :], xt.ap()[k * P : (k + 1) * P, :])
                wt = wpool.tile([P, 512], F16, name="wt")
                nc.sync.dma_start(wt[:], Wt.ap()[0, k, 0])
                w_pre.append(wt)

            gam = gbpool.tile([P, L, KT], F32, name="gam")
            bet = gbpool.tile([P, L, KT], F32, name="bet")
            for l in range(L):
                nc.scalar.dma_start(gam[:, l, :], gamma.ap()[l])
                nc.scalar.dma_start(bet[:, l, :], beta.ap()[l])

            sumexp = small.tile([P, KT], F32, name="sumexp")
            sum2 = small.tile([P, 8], F32, name="sum2")
            # last layer's final supertile weights, preloaded for the
            # tile-major pass
            wt7 = w7pool.tile([P, KT, 512], F16, name="wt7")

            for l in range(L):
                last = l == L - 1
                src = h[l % 2]
                dst = h[(l + 1) % 2]

                stat6 = small.tile([P, KT, 6], F32, name=f"stat6_{l}")
                meanvar = small.tile([P, KT, 2], F32, name=f"meanvar_{l}")

                # ---- matmul phase: out^T[n, m] = sum_k W[k, n] * h^T[k, m]
                ps_hold = {}
                n_seq = NSUP - 1 if last else NSUP
                for ns in range(n_seq):
                    ps = [psum.tile([P, BM], F32, name="ps") for _ in range(4)]
                    for k in range(KT):
                        if l == 0 and ns == 0:
                            wt = w_pre[k]
                        else:
                            wt = wpool.tile([P, 512], F16, name="wt")
                            nc.sync.dma_start(wt[:], Wt.ap()[l, k, ns])
                        for j in range(4):
                            nc.tensor.matmul(
                                ps[j][:],
                                wt[:, j * P : (j + 1) * P],
                                src[:, k, :],
                                start=(k == 0),
                                stop=(k == KT - 1),
                            )
                    if last and ns == 4:
                        # preload ns=7's weights for the tile-major pass
                        for k in range(KT):
                            nc.sync.dma_start(wt7[:, k, :], Wt.ap()[l, k, 7])
                    # On the last layer, tiles 24..31 skip the pre-BN copy:
                    # the exp-apply reads straight from PSUM (nothing needs
                    # those banks afterwards).
                    hold = last and ns == 6
                    for j in range(4):
                        t = ns * 4 + j
                        nc.vector.bn_stats(stat6[:, t, :], ps[j][:])
                        nc.vector.bn_aggr(meanvar[:, t, :], stat6[:, t, :])
                        if not hold:
                            nc.vector.tensor_copy(dst[:, t, :], ps[j][:])
                    if hold:
                        ps_hold[6] = ps
                if last:
                    # ns=7 tile-major: each tile's 32-step k-loop runs to
                    # completion so its BN stats start while the next tile's
                    # matmuls run; only tile 31's stats trail the last matmul.
                    ps = [psum.tile([P, BM], F32, name="ps") for _ in range(4)]
                    for j in range(4):
                        t = 28 + j
                        for k in range(KT):
                            nc.tensor.matmul(
                                ps[j][:],
                                wt7[:, k, j * P : (j + 1) * P],
                                src[:, k, :],
                                start=(k == 0),
                                stop=(k == KT - 1),
                            )
                        nc.vector.bn_stats(stat6[:, t, :], ps[j][:])
                        nc.vector.bn_aggr(meanvar[:, t, :], stat6[:, t, :])
                    ps_hold[7] = ps

                if not last:
                    # ---- BN: chunked cross-core mean / E[h^2] allreduce +
                    # fused scale/shift/relu apply
                    for ci, (t0, t1) in enumerate(CHUNKS):
                        n = t1 - t0
                        tag = f"{l}_{ci}"
                        pack = _pack_stats(nc, small, meanvar, t0, t1, tag)
                        red = small.tile([P, 2, n], F32, name=f"red_{tag}")
                        allreduce(dram, pack[:], red, tag)
                        scale, shift = _bn_scale_shift(
                            nc, small, red, gam[:, l, t0:t1], bet[:, l, t0:t1], n, tag
                        )
                        for i in range(n):
                            t = t0 + i
                            nc.scalar.activation(
                                dst[:, t, :],
                                dst[:, t, :],
                                mybir.ActivationFunctionType.Relu,
                                bias=shift[:, i : i + 1],
                                scale=scale[:, i : i + 1],
                            )
                else:
                    # ---- last layer: BN + exp + batch-axis softmax.
                    # exp(relu(z)) = max(exp(z), 1); the DVE max also
                    # accumulates the per-channel exp-sum for the softmax
                    # denominator.
                    # Early chunks, tiles [0,24): BN-stat allreduce + exp.
                    for ci, (t0, t1) in enumerate(CHUNKS_LAST):
                        n = t1 - t0
                        tag = f"L_{ci}"
                        pack = _pack_stats(nc, small, meanvar, t0, t1, tag)
                        red = small.tile([P, 2, n], F32, name=f"red_{tag}")
                        allreduce(dram, pack[:], red, tag)
                        scale, shift = _bn_scale_shift(
                            nc, small, red, gam[:, l, t0:t1], bet[:, l, t0:t1], n, tag
                        )
                        for i in range(n):
                            t = t0 + i
                            nc.scalar.activation(
                                dst[:, t, :],
                                dst[:, t, :],
                                mybir.ActivationFunctionType.Exp,
                                bias=shift[:, i : i + 1],
                                scale=scale[:, i : i + 1],
                            )
                            nc.vector.tensor_scalar(
                                dst[:, t, :],
                                dst[:, t, :],
                                1.0,
                                0.0,
                                mybir.AluOpType.max,
                                mybir.AluOpType.add,
                                accum_out=sumexp[:, t : t + 1],
                            )

                    # Softmax-sum allreduce for tiles [0,24): issued as soon
                    # as their exp accumulation finishes -- well before the
                    # last matmul, on an otherwise idle CC core.
                    redSA = small.tile([P, 24], F32, name="redSA")
                    allreduce(dram, sumexp[:, 0:24], redSA, "sumA")
                    rsumA = small.tile([P, 24], F32, name="rsumA")
                    nc.vector.reciprocal(rsumA[:], redSA[:])

                    # BN-stat allreduce for tiles [24,32): the first of the
                    # two small post-matmul collectives.
                    packB = _pack_stats(nc, small, meanvar, 24, 32, "sB")
                    redB = small.tile([P, 2, 8], F32, name="red_sB")
                    allreduce(dram, packB[:], redB, "sB", eng=nc.scalar)

                    # Normalize + store tiles [0,24): everything is ready the
                    # moment the last matmul retires, so this hides under the
                    # [24,32) stat allreduce.
                    for t in range(24):
                        nc.vector.tensor_scalar_mul(
                            dst[:, t, :], dst[:, t, :], rsumA[:, t : t + 1]
                        )
                        nc.sync.dma_start(
                            outt.ap()[t * P : (t + 1) * P, :], dst[:, t, :]
                        )

                    # Critical tail chain: BN scale/shift for tiles [24,32),
                    # exp from PSUM, second-stage sum allreduce.
                    scaleB, shiftB = _bn_scale_shift(
                        nc, small, redB, gam[:, l, 24:32], bet[:, l, 24:32], 8, "sB"
                    )
                    for t in range(24, KT):
                        ns, j = t // 4, t % 4
                        i = t - 24
                        nc.scalar.activation(
                            dst[:, t, :],
                            ps_hold[ns][j][:],
                            mybir.ActivationFunctionType.Exp,
                            bias=shiftB[:, i : i + 1],
                            scale=scaleB[:, i : i + 1],
                        )
                        nc.vector.tensor_scalar(
                            dst[:, t, :],
                            dst[:, t, :],
                            1.0,
                            0.0,
                            mybir.AluOpType.max,
                            mybir.AluOpType.add,
                            accum_out=sum2[:, i : i + 1],
                        )
                    # Final small allreduce: softmax sums of tiles [24,32).
                    red2 = small.tile([P, 8], F32, name="red2")
                    allreduce(dram, sum2[:], red2, "s2", eng=nc.scalar)
                    rsumB = small.tile([P, 8], F32, name="rsumB")
                    nc.vector.reciprocal(rsumB[:], red2[:])
                    for t in range(24, KT):
                        i = t - 24
                        nc.vector.tensor_scalar_mul(
                            dst[:, t, :], dst[:, t, :], rsumB[:, i : i + 1]
                        )
                        nc.sync.dma_start(
                            outt.ap()[t * P : (t + 1) * P, :], dst[:, t, :]
                        )

    nc.compile()
    _cached_nc = nc
    return nc


def make_in_maps(x, W, gamma, beta):
    """Host-side prep: shard x over the batch dim, transpose to [D, BM],
    convert the matmul operands to fp16 (weights also retiled so each
    [P, 512] tile is contiguous), transpose gamma/beta to [L, P, KT]."""
    x = np.asarray(x, dtype=np.float32)
    W = np.asarray(W, dtype=np.float32)
    gamma = np.asarray(gamma, dtype=np.float32)
    beta = np.asarray(beta, dtype=np.float32)
    # W[l, k*P+p, ns*512+c] -> Wtiled[l, k, ns, p, c]
    Wtiled = np.empty((L, KT, NSUP, P, 512), dtype=np.float16)
    Wtiled[...] = np.ascontiguousarray(W).reshape(L, KT, P, NSUP, 512).transpose(
        0, 1, 3, 2, 4
    )
    # [L, D] -> [L, P, KT]: channel (t*128 + p) lands at [l, p, t]
    gammaH = np.ascontiguousarray(gamma.reshape(L, KT, P).transpose(0, 2, 1))
    betaH = np.ascontiguousarray(beta.reshape(L, KT, P).transpose(0, 2, 1))
    in_maps = []
    for c in range(N_CORES):
        xt_c = np.ascontiguousarray(x[c * BM : (c + 1) * BM, :].T.astype(np.float16))
        in_maps.append(
            {"xt": xt_c, "W": Wtiled, "gammaH": gammaH, "betaH": betaH}
        )
    return in_maps


def kernel(x, W, b, gamma, beta):
    """Full (unsharded) inputs -> full [4096, 4096] softmax output."""
    del b  # canceled by BatchNorm mean subtraction
    nc = build()
    in_maps = make_in_maps(x, W, gamma, beta)
    r = bass_utils.run_bass_kernel_spmd(nc, in_maps, core_ids=list(range(N_CORES)))
    out = np.empty((N_CORES * BM, D), dtype=np.float32)
    for c in range(N_CORES):
        out[c * BM : (c + 1) * BM, :] = r.results[c]["outt"].T.astype(np.float32)
    return out
